# revision 50
# baseline (speedup 1.0000x reference)
"""BiGCN (two fused GCNConv + graph mean-pool + FC + log_softmax) on 8 trn2 cores.

Strategy (graph/data parallel, partitioned by destination node range):
  - core c owns nodes [c*NSH, (c+1)*NSH) as edge destinations
  - host precomputes dinv = 1/sqrt(deg+1) and the transposed x shard, so the
    device does no degree counting and no PE transposes in phase 1
  - device P1: Hn = (xT @ [W_td | W_bu]) * dinv (bf16), computed in batched
    x chunks, kept resident in SBUF, and AllGathered in 4 bank stripes
    (banked so gather indices fit int16)
  - host sorts edges into merged (4-tile-group, bank) gather cells with
    per-core COMPACTED index lists (pads only trail, skipped by the -1
    trim); the per-(tile, bank) one-hots carry tile-local dst rows in
    duplicated boundary-chunk columns, so a 128-slot chunk may span tiles
  - gather descriptor generation is spread over all 4 SWDGE queues (one Q7
    core pair each, ~10 ns/row/pair); the first WIN groups are emitted
    bank-major so each bank's gathers dispatch as soon as its AllGather
    lands; Tile's DMASW lane sems require queue = emission-index % 4 and
    dummy gathers pad the rotation so the (queue-0) indirect scatters land
    on queue-0 lanes
  - per dst tile: one-hot matmul scatter into PSUM, self-loop via identity
    matmul, out = dinv*(sum + Hn) + b; feat = [relu(td),td,relu(bu),bu,1]
  - graph pooling via one-hot matmul (built with a broadcast tensor_tensor),
    indirect-scatter into a bf16 partial, bf16 AllReduce, FC + log_softmax
    replicated on every core.
"""

import math

import numpy as np
import ml_dtypes

import concourse.bass as bass
import concourse.bacc as bacc
import concourse.mybir as mybir
import concourse.tile as tile
from concourse.bass import IndirectOffsetOnAxis
from concourse.bass_utils import run_bass_kernel_spmd
from concourse.library_config import mlp as mlp_lib

BF16 = mybir.dt.bfloat16
F32 = mybir.dt.float32
I16 = mybir.dt.int16
I32 = mybir.dt.int32
AF = mybir.ActivationFunctionType
ALU = mybir.AluOpType
NPBF = ml_dtypes.bfloat16

P = 128  # partitions / tile height

PREP_MODE = False  # prepare_only + trigger_dma pipeline (False: direct gathers)
LEAD = 8  # tiles of desc-gen lead ahead of triggers
PARTIAL_BF16 = True  # pooled partial + AllReduce in bf16
GROUP = 4  # dst tiles merged into one gather instruction per bank
WIN_GROUPS = 4  # gather lookahead window, in groups


def _split_even(n, k):
    base = n // k
    rem = n % k
    return [base + (1 if i < rem else 0) for i in range(k)]


def _split_progressive(n, k, cap):
    """Small first stripes, later stripes at the int16 cap: the first
    AllGather launches after only a few P1 tiles, so the gather stream
    starts early and the collective chain hides under P1 + early gathers."""
    sizes = [0] * k
    rem = n
    for i in range(k - 1, -1, -1):
        take = min(cap, rem - i)
        sizes[i] = take
        rem -= take
    assert rem == 0 and all(s >= 1 for s in sizes)
    return sizes


class Cfg:
    def __init__(self, n_nodes, n_graphs, n_cores, banks, in_f, hid_f, out_f):
        assert n_nodes % n_cores == 0
        self.N = n_nodes
        self.G = n_graphs
        self.NC = n_cores
        self.NSH = n_nodes // n_cores  # nodes per core
        self.T = math.ceil(self.NSH / P)  # dst tiles per core
        self.NSH_P = self.T * P  # padded shard rows
        self.BANKS = min(banks, self.T)
        # bank k holds the stripe of tiles [qt_start[k], qt_start[k+1]) from
        # every core: bank rows = NC * qrows[k]
        # even stripes beat a progressive (small-first) split on HW: tiny
        # early banks starve the Q7 pairs before the next AllGather lands,
        # and max-size late banks coarsen cells and lengthen their AGs
        self.QT = _split_even(self.T, self.BANKS)
        self.QSTART = np.concatenate([[0], np.cumsum(self.QT)])  # tile starts
        self.QROWS = [q * P for q in self.QT]
        for k in range(self.BANKS):
            assert self.NC * self.QROWS[k] <= 32767, "bank idx must fit int16"
        self.IN_F = in_f
        self.HID = hid_f
        self.FW = 2 * hid_f
        assert self.FW == P and in_f == P
        self.OUT_F = out_f
        self.FEAT = 4 * hid_f + 1
        self.GB = math.ceil(self.G / P)
        # rows needed: max goff1+127 = (G-1) + 255
        self.PART_ROWS = math.ceil((self.G - 1 + 2 * P) / P) * P


def host_prep(cfg, x, edge_index, batch):
    """Build per-core edge grids + constants. Returns (meta, per_core_inputs)."""
    c = cfg
    src = edge_index[0].astype(np.int64)
    dst = edge_index[1].astype(np.int64)
    assert src.min() >= 0 and src.max() < c.N and dst.min() >= 0 and dst.max() < c.N

    qstart_rows = c.QSTART[:-1] * P  # local row where each bank stripe starts
    sc = src // c.NSH  # owner core of src
    so = src % c.NSH  # local row of src
    stile = so // P
    bank = np.searchsorted(c.QSTART[1:], stile, side="right")
    lidx = sc * np.asarray(c.QROWS)[bank] + (so - qstart_rows[bank])

    owner = dst // c.NSH
    tloc = (dst % c.NSH) // P
    dl = ((dst % c.NSH) % P).astype(np.int64)

    ncell = c.NC * c.T * c.BANKS
    cell = (owner * c.T + tloc) * c.BANKS + bank
    order = np.argsort(cell, kind="stable")
    cell_s = cell[order]
    lidx_s = lidx[order]
    dl_s = dl[order]
    counts = np.bincount(cell_s, minlength=ncell).reshape(c.NC, c.T, c.BANKS)
    starts = np.zeros(ncell + 1, dtype=np.int64)
    np.cumsum(counts.reshape(-1), out=starts[1:])

    NGRP = math.ceil(c.T / GROUP)

    # host-side degree/dinv (deg includes the self loop)
    deg = np.bincount(dst, minlength=c.N).astype(np.float64) + 1.0
    dinv = (1.0 / np.sqrt(deg)).astype(np.float32)

    g_base = np.empty(c.NC, dtype=np.int64)
    for cc in range(c.NC):
        b = batch[cc * c.NSH : (cc + 1) * c.NSH]
        g_base[cc] = int(b[0])
        assert int(b[-1]) - int(b[0]) < 2 * P, "graph span exceeds 2 blocks"

    # merged (group, bank) gather cells: per-core edges of the group's tiles
    # are COMPACTED (pads only trail, skipped by the -1 trim). A 128-slot
    # chunk may span two tiles; the per-(t, j) one-hot uses group-local dst
    # rows (toff*128 + row) vs an iota512 window to mask foreign slots.
    # Chunk ranges per (t, j) are the union over cores.
    ngc = np.zeros((c.NC, NGRP, c.BANKS), dtype=np.int64)  # edges per core
    for g in range(NGRP):
        t0, t1 = g * GROUP, min((g + 1) * GROUP, c.T)
        ngc[:, g, :] = counts[:, t0:t1, :].sum(axis=1)
    Bg = {}  # (g, j) -> chunks of the merged cell (max over cores)
    ecb = {}  # (g, j) -> eidx chunk col base
    col = 0
    for g in range(NGRP):
        for j in range(c.BANKS):
            ecb[(g, j)] = col
            Bg[(g, j)] = int(-(-(max(int(ngc[:, g, j].max()), 1)) // P))
            col += Bg[(g, j)]
    ECOLS = col * 8
    NMG = NGRP * c.BANKS

    # union chunk range of tile t inside its merged cell, across cores
    crange = {}  # (t, j) -> (clo, chi) chunk indices within the cell
    for g in range(NGRP):
        t0, t1 = g * GROUP, min((g + 1) * GROUP, c.T)
        for j in range(c.BANKS):
            cum = np.zeros(c.NC, dtype=np.int64)
            for t in range(t0, t1):
                nt = counts[:, t, j]
                lo = int(cum.min()) // P
                if int(nt.max()) > 0:
                    hi = -(-int((cum + nt).max()) // P)
                    crange[(t, j)] = (lo, hi)
                else:
                    crange[(t, j)] = (lo, lo)
                cum += nt

    # dl columns ordered (t, j, chunk-in-range); boundary chunks duplicate
    dl_col = {}
    col = 0
    for t in range(c.T):
        for j in range(c.BANKS):
            dl_col[(t, j)] = col
            lo, hi = crange[(t, j)]
            col += hi - lo
    GCH = max(col, 1)

    per_core = []
    scrub_lo = {}  # (g, j) -> lowest chunk any core leaves partially unwritten
    for cc in range(c.NC):
        eidx = np.zeros((P, max(ECOLS, 8)), dtype=np.int16)
        dlh = np.full((P, GCH), 4096.0, dtype=np.float32)
        nreal = np.zeros((1, NMG), dtype=np.int32)
        for g in range(NGRP):
            t0, t1 = g * GROUP, min((g + 1) * GROUP, c.T)
            for j in range(c.BANKS):
                B = Bg[(g, j)]
                slots = B * P
                li = np.full(slots, -1, dtype=np.int64)
                dv = np.full(slots, 4096.0, dtype=np.float64)
                pos = 0
                for t in range(t0, t1):
                    ci = (cc * c.T + t) * c.BANKS + j
                    s0, s1 = starts[ci], starts[ci + 1]
                    n = int(s1 - s0)
                    li[pos : pos + n] = lidx_s[s0:s1]
                    dv[pos : pos + n] = (t - t0) * P + dl_s[s0:s1]
                    pos += n
                if pos == 0:
                    li[0] = 0  # keep >=1 real idx (sim/ucode edge case)
                    pos = 1
                nreal[0, g * c.BANKS + j] = pos
                scrub_lo[(g, j)] = min(scrub_lo.get((g, j), 1 << 30), pos // P)
                w = li.reshape(slots // 16, 16).T.astype(np.int16)
                eb = ecb[(g, j)]
                eidx[:, eb * 8 : eb * 8 + B * 8] = np.tile(w, (8, 1))
                dvc = dv.reshape(B, P).T  # [P, B] chunk columns
                for t in range(t0, t1):
                    lo, hi = crange[(t, j)]
                    if hi > lo:
                        cb = dl_col[(t, j)]
                        # tile-local values: own edges land in [0,128) (bf16
                        # exact); foreign/pad values stay far outside even
                        # after bf16 rounding, so is_equal vs iota128 is safe
                        dlh[:, cb : cb + hi - lo] = (
                            dvc[:, lo:hi] - (t - t0) * P
                        )

        xs = np.zeros((c.NSH_P, c.IN_F), dtype=np.float32)
        xs[: c.NSH] = x[cc * c.NSH : (cc + 1) * c.NSH]
        xT = np.ascontiguousarray(xs.T)  # [IN_F, NSH_P]

        dv = np.ones(c.T * P, dtype=np.float32)
        dv[: c.NSH] = dinv[cc * c.NSH : (cc + 1) * c.NSH]
        dinvT = np.ascontiguousarray(dv.reshape(c.T, P).T)  # [P, T]

        brel = np.full(c.T * P, 60000.0, dtype=np.float32)
        brel[: c.NSH] = batch[cc * c.NSH : (cc + 1) * c.NSH] - g_base[cc]
        batchT = brel.reshape(c.T, P).T.astype(np.float32)

        goff0 = (g_base[cc] + np.arange(P)).astype(np.int32).reshape(P, 1)
        goff1 = goff0 + P
        per_core.append(
            dict(xT_sh=xT, dinvT=dinvT, eidx=eidx, dlh=dlh.astype(NPBF),
                 batchT=batchT, goff0=goff0, goff1=goff1, nreal=nreal)
        )

    iota512 = np.tile(
        np.arange(GROUP * P, dtype=np.float32), (P, 1)
    ).astype(NPBF)
    iota256 = np.tile(np.arange(2 * P, dtype=np.float32), (P, 1)).astype(NPBF)
    ident = np.eye(P, dtype=np.float32).astype(NPBF)
    consts = dict(iota512=iota512, iota256=iota256, ident=ident)
    meta = dict(GCH=GCH, ECOLS=max(ECOLS, 8), consts=consts,
                dl_col=dl_col, ecb=ecb, crange=crange, Bg=Bg, NGRP=NGRP,
                NMG=NMG, scrub_lo=scrub_lo)
    return meta, per_core


def build_program(cfg, meta, debug=False):
    c = cfg
    GCH = meta["GCH"]
    ECOLS = meta["ECOLS"]
    dl_col = meta["dl_col"]
    ecb = meta["ecb"]
    crange = meta["crange"]
    scrub_lo = meta["scrub_lo"]
    Bg = meta["Bg"]
    NGRP = meta["NGRP"]
    NMG = meta["NMG"]
    H = c.HID
    PD = BF16 if PARTIAL_BF16 else F32
    LEAD_G = max(1, LEAD // GROUP)  # gather lead in groups (prep mode)
    WIN = WIN_GROUPS  # gather window, in groups
    GAT_BUFS = WIN * c.BANKS  # in-flight merged gather tiles
    BMAX = max(Bg.values()) if Bg else 1  # widest merged cell (chunks)

    nc = bacc.Bacc(
        "TRN2", target_bir_lowering=False, debug=debug, num_devices=c.NC,
        num_swdge_queues=min(4, c.BANKS),
    )

    # ---- I/O ----
    xT_sh = nc.dram_tensor("xT_sh", [c.IN_F, c.NSH_P], F32, kind="ExternalInput")
    dinvT = nc.dram_tensor("dinvT", [P, c.T], F32, kind="ExternalInput")
    W_td = nc.dram_tensor("W_td", [c.IN_F, H], F32, kind="ExternalInput")
    W_bu = nc.dram_tensor("W_bu", [c.IN_F, H], F32, kind="ExternalInput")
    b_td = nc.dram_tensor("b_td", [H], F32, kind="ExternalInput")
    b_bu = nc.dram_tensor("b_bu", [H], F32, kind="ExternalInput")
    fc_W = nc.dram_tensor("fc_W", [4 * H, c.OUT_F], F32, kind="ExternalInput")
    fc_b = nc.dram_tensor("fc_b", [c.OUT_F], F32, kind="ExternalInput")
    eidx = nc.dram_tensor("eidx", [P, ECOLS], I16, kind="ExternalInput")
    dlh = nc.dram_tensor("dlh", [P, max(GCH, 1)], BF16, kind="ExternalInput")
    batchT = nc.dram_tensor("batchT", [P, c.T], F32, kind="ExternalInput")
    goff0 = nc.dram_tensor("goff0", [P, 1], I32, kind="ExternalInput")
    goff1 = nc.dram_tensor("goff1", [P, 1], I32, kind="ExternalInput")
    nreal = nc.dram_tensor("nreal", [1, NMG], I32, kind="ExternalInput")
    iota512_in = nc.dram_tensor(
        "iota512", [P, GROUP * P], BF16, kind="ExternalInput"
    )
    iota256_in = nc.dram_tensor("iota256", [P, 2 * P], BF16, kind="ExternalInput")
    ident_in = nc.dram_tensor("ident", [P, P], BF16, kind="ExternalInput")
    out = nc.dram_tensor("out", [c.G, c.OUT_F], F32, kind="ExternalOutput")

    # ---- internal DRAM ----
    hn_local = nc.dram_tensor("hn_local", [c.NSH_P, c.FW], BF16)
    hn_q = [
        nc.dram_tensor(f"hn_q{k}", [c.NC * c.QROWS[k], c.FW], BF16,
                       addr_space="Shared")
        for k in range(c.BANKS)
    ]
    partial = nc.dram_tensor("partial", [c.PART_ROWS, c.FEAT], PD)
    total = nc.dram_tensor("total", [c.PART_ROWS, c.FEAT], PD, addr_space="Shared")

    groups = [list(range(c.NC))]
    NQ = min(4, c.BANKS)  # swdge queues in use (queue = bank % NQ)

    with tile.TileContext(nc) as tc:
        with (
            tc.tile_pool(name="const", bufs=1) as cp,
            tc.tile_pool(name="sb", bufs=3) as sp,
            tc.tile_pool(name="ohb", bufs=2) as op_,
            tc.tile_pool(name="gat", bufs=WIN_GROUPS * c.BANKS) as gp,
            nc.gpsimd.register("nr0") as r0,
            nc.gpsimd.register("nr1") as r1,
            nc.gpsimd.register("nr2") as r2,
            nc.gpsimd.register("nr3") as r3,
        ):
            regs = [r0, r1, r2, r3]
            nc.gpsimd.load_library(mlp_lib)

            dsem = [nc.alloc_semaphore(f"gdma_q{j}") for j in range(NQ)]

            # ---- constants ----
            # P1-critical loads go on the sync HWDGE queue; bulk loads that
            # are only needed by the gather phase ride the scalar HWDGE
            # queue so they don't delay the first x chunks / AllGather
            wtmp = cp.tile([P, c.FW], F32)
            nc.sync.dma_start(wtmp[:, 0:H], W_td[:])
            nc.sync.dma_start(wtmp[:, H : 2 * H], W_bu[:])
            wcat = cp.tile([P, c.FW], BF16)
            nc.scalar.activation(wcat[:], wtmp[:], AF.Copy)
            dinv_sb = cp.tile([P, c.T], F32)
            nc.sync.dma_start(dinv_sb[:], dinvT[:])

            eidx_sb = cp.tile([P, ECOLS], I16)
            nc.scalar.dma_start(eidx_sb[:], eidx[:])

            iota_sb = cp.tile([P, GROUP * P], BF16)
            iota256_sb = cp.tile([P, 2 * P], BF16)
            ident_sb = cp.tile([P, P], BF16)
            nc.scalar.dma_start(iota_sb[:], iota512_in[:])
            nc.scalar.dma_start(iota256_sb[:], iota256_in[:])
            nc.scalar.dma_start(ident_sb[:], ident_in[:])
            ident32_sb = cp.tile([P, P], F32)
            nc.scalar.activation(ident32_sb[:], ident_sb[:], AF.Copy)

            btmp = cp.tile([1, c.FW], F32)
            nc.scalar.dma_start(btmp[0:1, 0:H], b_td[None, :])
            nc.scalar.dma_start(btmp[0:1, H : 2 * H], b_bu[None, :])
            bcat = cp.tile([1, c.FW], BF16)
            nc.scalar.activation(bcat[:], btmp[:], AF.Copy)
            ones_row = cp.tile([1, P], BF16)
            nc.vector.memset(ones_row[:], 1.0)
            bias_sb = cp.tile([P, c.FW], F32)

            dl_sb = cp.tile([P, max(GCH, 1)], BF16)
            nc.scalar.dma_start(dl_sb[:], dlh[:])
            nreal_sb = cp.tile([1, NMG], I32)
            nc.scalar.dma_start(nreal_sb[:], nreal[:])

            fw0 = cp.tile([P, c.OUT_F], F32)
            fw1 = cp.tile([P, c.OUT_F], F32)
            fcb = cp.tile([c.OUT_F, 1], F32)
            batch_sb = cp.tile([P, c.T], F32)
            goff0_sb = cp.tile([P, 1], I32)
            goff1_sb = cp.tile([P, 1], I32)
            hn_all = cp.tile([P, c.T * P], BF16)  # resident Hn tiles
            zt = sp.tile([P, c.FEAT], PD, tag="zt")

            def emit_late_loads():
                # needed only by the pooling epilogue / FC — emitted after
                # the gather window so they don't delay phase 1
                nc.scalar.dma_start(fw0[:], fc_W[0:P, :])
                nc.scalar.dma_start(fw1[:], fc_W[P : 2 * P, :])
                nc.scalar.dma_start(fcb[:, 0:1], fc_b[:, None])
                nc.scalar.dma_start(batch_sb[:], batchT[:])
                nc.scalar.dma_start(goff0_sb[:], goff0[:])
                nc.scalar.dma_start(goff1_sb[:], goff1[:])
                nc.vector.memset(zt[:], 0.0)
                for r in range(0, c.PART_ROWS, P):
                    nc.scalar.dma_start(partial[r : r + P, :], zt[:])

            # ---- gather machinery (one merged gather per (group, bank)) ----
            gts = {}  # (g, j) -> gather output tile
            prep_done = cp.tile([P, 8], F32)  # all-desc-gen-done marker
            acnt = [0]  # allocation counter (first-use scrub)
            ecnt = [0]  # emission counter (queue + reg round-robin)

            def alloc_cell(g, j):
                # pool slots are assigned in allocation order — keep that
                # aligned with (g, j) consumption order so slot WAR waits
                # stay one window behind
                B = int(Bg[(g, j)])
                if B == 0:
                    return
                # uniform-width tiles: slot reuse must never expose SBUF the
                # first-use scrub didn't cover (trailing pad slots are
                # skipped by the gather's -1 trim and reach the matmuls
                # zero-weighted, so they must be finite, not stale NaNs)
                gt_t = gp.tile([P, BMAX * P], BF16, tag="gt")
                gts[(g, j)] = gt_t
                if acnt[0] < GAT_BUFS:
                    # first slot use: whole tile may be NaN garbage
                    nc.vector.memset(gt_t[:], 0.0)
                else:
                    # the -1 trim leaves slots beyond this core's real count
                    # unwritten; scrub from the lowest chunk any core can
                    # leave partial up to this cell's width (stale finite
                    # data would be fine, but slot-width variance across
                    # reuses can expose never-written SBUF)
                    lo = min(scrub_lo[(g, j)], B - 1)
                    nc.vector.memset(gt_t[:, lo * P : B * P], 0.0)
                acnt[0] += 1

            def gather_cell(g, j):
                B = int(Bg[(g, j)])
                if B == 0:
                    return
                q = ecnt[0] % NQ
                gt_t = gts[(g, j)]
                eb = ecb[(g, j)]
                mg = g * c.BANKS + j
                reg = regs[ecnt[0] % 4]
                nc.gpsimd.reg_load(reg, nreal_sb[0:1, mg : mg + 1])
                kwargs = dict(queue_num=q, single_packet=(B * P <= 1024))
                if PREP_MODE:
                    kwargs.update(prepare_only=True, sem=dsem[q])
                nc.gpsimd.dma_gather(
                    gt_t[:, 0 : B * P].rearrange("p (b e) -> p b e", e=P),
                    hn_q[j][:],
                    eidx_sb[:, eb * 8 : eb * 8 + B * 8],
                    B * P,
                    reg,
                    c.FW,
                    **kwargs,
                )
                ecnt[0] += 1

            def emit_preps(g):
                for j in range(c.BANKS):
                    alloc_cell(g, j)
                    gather_cell(g, j)

            def emit_triggers(g, tail=False):
                if not PREP_MODE:
                    return
                if tail:
                    # tail triggers have no pending preps, so no nosync deps
                    # hold them in place — pin the schedule order explicitly
                    tc.no_sync_barrier()
                for j in range(c.BANKS):
                    if Bg[(g, j)] == 0:
                        continue
                    q = j % NQ
                    if tail:
                        # WAW on prep_done (written by the all-cores gpsimd
                        # memset after the last prep) orders the trigger
                        # after every pair's desc-gen has completed
                        nc.gpsimd.trigger_dma(
                            count=1, queue_num=q,
                            signals_writable=[prep_done[:]],
                        )
                    else:
                        # count=None fires the FIFO head; Tile attaches the
                        # pending prep's engine-completion wait + deferred
                        # data deps (hn_q AllGather, gt-slot WAR)
                        nc.gpsimd.trigger_dma(count=None, queue_num=q)

            # ---- prologue: desc-gen for the first LEAD_G groups ----
            # (prep mode only: direct gathers carry data deps on the
            # AllGathers, which are emitted later in the Pool stream —
            # hoisting them here would deadlock the in-order sequencer)
            if PREP_MODE:
                for g in range(min(LEAD_G, NGRP)):
                    emit_preps(g)
                if NGRP <= LEAD_G:
                    nc.gpsimd.memset(prep_done[:], 0.0)

            # ---- P1: Hn = (xT @ wcat) * dinv, then banked AllGathers ----
            CH = min(8, c.T)  # tiles per x chunk
            with (
                tc.tile_pool(name="p1x", bufs=2) as xp,
                tc.tile_pool(name="ps1", bufs=4, space="PSUM") as pp,
            ):
                bias_ps = pp.tile([P, c.FW], F32, space="PSUM", tag="bias")
                nc.tensor.matmul(
                    bias_ps[:], lhsT=ones_row[0:1, :], rhs=bcat[0:1, :],
                    start=True, stop=True,
                )
                nc.vector.tensor_copy(bias_sb[:], bias_ps[:])

                for k in range(c.BANKS):
                    for t0 in range(int(c.QSTART[k]), int(c.QSTART[k + 1]), CH):
                        t1 = min(t0 + CH, int(c.QSTART[k + 1]))
                        w = (t1 - t0) * P
                        xf = xp.tile([P, CH * P], F32, tag="xf")
                        nc.sync.dma_start(xf[:, 0:w], xT_sh[:, t0 * P : t1 * P])
                        xb = xp.tile([P, CH * P], BF16, tag="xb")
                        nc.scalar.activation(xb[:, 0:w], xf[:, 0:w], AF.Copy)
                        for t in range(t0, t1):
                            h_ps = pp.tile([P, c.FW], F32, space="PSUM", tag="h")
                            nc.tensor.matmul(
                                h_ps[:],
                                lhsT=xb[:, (t - t0) * P : (t - t0 + 1) * P],
                                rhs=wcat[:],
                                start=True, stop=True,
                            )
                            nc.scalar.activation(
                                hn_all[:, t * P : (t + 1) * P], h_ps[:],
                                AF.Copy, scale=dinv_sb[:, t : t + 1],
                            )
                            nc.sync.dma_start(
                                hn_local[t * P : (t + 1) * P, :],
                                hn_all[:, t * P : (t + 1) * P],
                            )

                    r_lo = int(c.QSTART[k]) * P
                    nc.gpsimd.collective_compute(
                        "AllGather",
                        ALU.bypass,
                        ins=[hn_local[r_lo : r_lo + c.QROWS[k], :]],
                        outs=[hn_q[k][:]],
                        replica_groups=groups,
                    )

            # ---- initial gather window (direct mode): bank-major so each
            # bank's first gathers dispatch as soon as its AllGather lands,
            # keeping all queues fed while later AllGathers finish ----
            if not PREP_MODE:
                for g in range(min(WIN, NGRP)):
                    for j in range(c.BANKS):
                        alloc_cell(g, j)
                for j in range(c.BANKS):
                    for g in range(min(WIN, NGRP)):
                        gather_cell(g, j)
            emit_late_loads()

            # ---- P4: trigger + scatter-add + feat + pooling ----
            def onehot_big(t, tag):
                # dl columns hold tile-local dst rows; foreign-tile and pad
                # slots sit far outside [0, 128) so they never match
                g0 = dl_col[(t, 0)]
                gt = sum(
                    crange[(t, j)][1] - crange[(t, j)][0]
                    for j in range(c.BANKS)
                )
                oh = op_.tile([P, gt * P], BF16, tag=tag)
                nc.vector.tensor_tensor(
                    out=oh[:].rearrange("p (g d) -> p g d", d=P),
                    in0=iota_sb[:, 0:P].unsqueeze(1).broadcast_to([P, gt, P]),
                    in1=dl_sb[:, g0 : g0 + gt].to_broadcast([P, gt, P]),
                    op=ALU.is_equal,
                )
                return oh, g0, gt

            with (
                tc.tile_pool(name="ps4", bufs=2, space="PSUM") as pp,
                tc.tile_pool(name="psacc", bufs=1, space="PSUM") as pa,
            ):
                pool_ps0 = pa.tile([P, c.FEAT], F32, space="PSUM")
                pool_ps1 = pa.tile([P, c.FEAT], F32, space="PSUM")
                for t in range(c.T):
                    g = t // GROUP
                    if t % GROUP == 0:
                        if PREP_MODE:
                            if g + LEAD_G < NGRP:
                                emit_preps(g + LEAD_G)
                                if g + LEAD_G == NGRP - 1:
                                    # marker after the final prep: the
                                    # all-cores memset completes only once
                                    # every Q7 pair finished its desc-gen
                                    nc.gpsimd.memset(prep_done[:], 0.0)
                            emit_triggers(g, tail=(g + LEAD_G >= NGRP))
                        elif g + WIN < NGRP:
                            emit_preps(g + WIN)

                    tcells = [
                        (j, dl_col[(t, j)], crange[(t, j)])
                        for j in range(c.BANKS)
                        if crange[(t, j)][1] > crange[(t, j)][0]
                    ]
                    nch = sum(hi - lo for _, _, (lo, hi) in tcells)
                    acc = pp.tile([P, c.FW], F32, space="PSUM", tag="acc")
                    if nch:
                        oh, g0, gtn = onehot_big(t, "ohb2")
                        kk = 0
                        for j, cb, (lo, hi) in tcells:
                            gt_t = gts[(g, j)]
                            for q in range(hi - lo):
                                nc.tensor.matmul(
                                    acc[:],
                                    lhsT=oh[:, (cb - g0 + q) * P
                                            : (cb - g0 + q + 1) * P],
                                    rhs=gt_t[:, (lo + q) * P
                                             : (lo + q + 1) * P],
                                    start=(kk == 0),
                                    stop=False,
                                )
                                kk += 1
                    nc.tensor.matmul(
                        acc[:], lhsT=ident_sb[:],
                        rhs=hn_all[:, t * P : (t + 1) * P],
                        start=(nch == 0), stop=True,
                    )

                    ot = sp.tile([P, c.FW], F32, tag="ot")
                    nc.scalar.activation(
                        ot[:], acc[:], AF.Copy, scale=dinv_sb[:, t : t + 1]
                    )
                    nc.vector.tensor_tensor(
                        out=ot[:], in0=ot[:], in1=bias_sb[:], op=ALU.add
                    )
                    feat = sp.tile([P, c.FEAT], BF16, tag="feat")
                    nc.scalar.activation(feat[:, 0:H], ot[:, 0:H], AF.Relu)
                    nc.scalar.copy(feat[:, H : 2 * H], ot[:, 0:H])
                    nc.scalar.activation(
                        feat[:, 2 * H : 3 * H], ot[:, H : 2 * H], AF.Relu
                    )
                    nc.scalar.copy(feat[:, 3 * H : 4 * H], ot[:, H : 2 * H])
                    nc.vector.memset(feat[:, 4 * H : 4 * H + 1], 1.0)

                    ohg = sp.tile([P, 2 * P], BF16, tag="ohg")
                    nc.vector.tensor_tensor(
                        out=ohg[:].rearrange("p (g d) -> p g d", d=2 * P),
                        in0=iota256_sb[:].unsqueeze(1),
                        in1=batch_sb[:, t : t + 1].to_broadcast([P, 1, 2 * P]),
                        op=ALU.is_equal,
                    )
                    nc.tensor.matmul(
                        pool_ps0[:], lhsT=ohg[:, 0:P], rhs=feat[:],
                        start=(t == 0), stop=(t == c.T - 1),
                    )
                    nc.tensor.matmul(
                        pool_ps1[:], lhsT=ohg[:, P : 2 * P], rhs=feat[:],
                        start=(t == 0), stop=(t == c.T - 1),
                    )

                # ---- P5: scatter local pooled windows ----
                # Tile's DMASW lane sems rotate over Pool DMA instructions
                # and each lane is locked to one SWDGE queue. The indirect
                # scatters are pinned to queue 0, so pad the rotation with
                # dummy gathers to land them on queue-0 lanes (0 and 4);
                # the barrier keeps the no-dep dummies from being hoisted.
                def dummy_gather(scrap):
                    tc.no_sync_barrier()
                    q = ecnt[0] % NQ
                    nc.gpsimd.dma_gather(
                        scrap[:].rearrange("p (b e) -> p b e", e=P),
                        hn_q[0][:],
                        eidx_sb[:, 0:8],
                        P,
                        P,
                        c.FW,
                        queue_num=q,
                        single_packet=True,
                    )
                    ecnt[0] += 1

                tc.no_sync_barrier()
                scrap = sp.tile([P, P], BF16, tag="scrap")
                while ecnt[0] % 4 != 0:
                    dummy_gather(scrap)
                pp0 = sp.tile([P, c.FEAT], PD, tag="pp0")
                nc.vector.tensor_copy(pp0[:], pool_ps0[:])
                tc.no_sync_barrier()
                nc.gpsimd.indirect_dma_start(
                    out=partial[:],
                    out_offset=IndirectOffsetOnAxis(ap=goff0_sb[:, 0:1], axis=0),
                    in_=pp0[:],
                    in_offset=None,
                )
                ecnt[0] += 1
                for _ in range(3):
                    dummy_gather(scrap)
                pp1 = sp.tile([P, c.FEAT], PD, tag="pp1")
                nc.vector.tensor_copy(pp1[:], pool_ps1[:])
                tc.no_sync_barrier()
                nc.gpsimd.indirect_dma_start(
                    out=partial[:],
                    out_offset=IndirectOffsetOnAxis(ap=goff1_sb[:, 0:1], axis=0),
                    in_=pp1[:],
                    in_offset=None,
                )

            # ---- P6: AllReduce pooled sums ----
            nc.gpsimd.collective_compute(
                "AllReduce",
                ALU.add,
                ins=[partial[:]],
                outs=[total[:]],
                replica_groups=groups,
            )

            # ---- P7: mean, FC, log_softmax (replicated) ----
            with (
                tc.tile_pool(name="ps7", bufs=2, space="PSUM") as pp,
                tc.tile_pool(name="sb7", bufs=4) as sp,
            ):
                for b in range(c.GB):
                    h_rows = min(P, c.G - b * P)
                    tt = sp.tile([P, c.FEAT], PD, tag="tt")
                    nc.sync.dma_start(tt[:], total[b * P : (b + 1) * P, :])
                    rec = sp.tile([P, 1], F32, tag="rec")
                    nc.vector.tensor_scalar(
                        out=rec[:], in0=tt[:, 4 * H : 4 * H + 1], scalar1=1.0,
                        scalar2=None, op0=ALU.max,
                    )
                    nc.vector.reciprocal(rec[:], rec[:])
                    mean_sb = sp.tile([P, 4 * H], F32, tag="mean")
                    nc.vector.tensor_scalar(
                        out=mean_sb[:], in0=tt[:, 0 : 4 * H],
                        scalar1=rec[:, 0:1], scalar2=None, op0=ALU.mult,
                    )
                    lg_ps = pp.tile([P, P], F32, space="PSUM", tag="lg")
                    for half in range(2):
                        tp_ps = pp.tile([P, P], F32, space="PSUM", tag="tp")
                        nc.tensor.transpose(
                            tp_ps[:], mean_sb[:, half * P : (half + 1) * P],
                            ident32_sb[:],
                        )
                        mt = sp.tile([P, P], F32, tag="mt")
                        nc.vector.tensor_copy(mt[:], tp_ps[:])
                        nc.tensor.matmul(
                            lg_ps[0 : c.OUT_F, :],
                            lhsT=(fw0 if half == 0 else fw1)[:],
                            rhs=mt[:],
                            start=(half == 0),
                            stop=(half == 1),
                        )
                    lgb = sp.tile([c.OUT_F, P], F32, tag="lgb")
                    nc.vector.tensor_scalar(
                        out=lgb[:], in0=lg_ps[0 : c.OUT_F, :],
                        scalar1=fcb[:, 0:1], scalar2=None, op0=ALU.add,
                    )
                    tr_ps = pp.tile([P, c.OUT_F], F32, space="PSUM", tag="tr")
                    nc.tensor.transpose(
                        tr_ps[:], lgb[:], ident32_sb[0 : c.OUT_F, 0 : c.OUT_F]
                    )
                    ls = sp.tile([P, c.OUT_F], F32, tag="ls")
                    nc.vector.tensor_copy(ls[:], tr_ps[:])
                    mx = sp.tile([P, 1], F32, tag="mx")
                    nc.vector.reduce_max(mx[:], ls[:], axis=mybir.AxisListType.X)
                    nc.vector.tensor_scalar(
                        out=ls[:], in0=ls[:], scalar1=mx[:, 0:1], scalar2=None,
                        op0=ALU.subtract,
                    )
                    ex = sp.tile([P, c.OUT_F], F32, tag="ex")
                    nc.scalar.activation(ex[:], ls[:], AF.Exp)
                    sm = sp.tile([P, 1], F32, tag="sm")
                    nc.vector.reduce_sum(sm[:], ex[:], axis=mybir.AxisListType.X)
                    nc.scalar.activation(sm[:], sm[:], AF.Ln)
                    nc.vector.tensor_scalar(
                        out=ls[:], in0=ls[:], scalar1=sm[:, 0:1], scalar2=None,
                        op0=ALU.subtract,
                    )
                    nc.sync.dma_start(
                        out[b * P : b * P + h_rows, :], ls[0:h_rows, :]
                    )

    nc.compile()
    return nc


def make_in_maps(cfg, meta, per_core, W_td, b_td, W_bu, b_bu, fc_W, fc_b):
    cst = meta["consts"]
    in_maps = []
    for cc in range(cfg.NC):
        pc = per_core[cc]
        in_maps.append(
            {
                "xT_sh": pc["xT_sh"],
                "dinvT": pc["dinvT"],
                "W_td": np.asarray(W_td, dtype=np.float32),
                "W_bu": np.asarray(W_bu, dtype=np.float32),
                "b_td": np.asarray(b_td, dtype=np.float32),
                "b_bu": np.asarray(b_bu, dtype=np.float32),
                "fc_W": np.asarray(fc_W, dtype=np.float32),
                "fc_b": np.asarray(fc_b, dtype=np.float32),
                "eidx": pc["eidx"],
                "dlh": pc["dlh"],
                "batchT": pc["batchT"],
                "goff0": pc["goff0"],
                "goff1": pc["goff1"],
                "nreal": pc["nreal"],
                "iota512": cst["iota512"],
                "iota256": cst["iota256"],
                "ident": cst["ident"],
            }
        )
    return in_maps


def prep_and_build(cfg, inputs, debug=False):
    x = np.asarray(inputs["x"], dtype=np.float32)
    edge_index = np.asarray(inputs["edge_index"])
    batch = np.asarray(inputs["batch"]).astype(np.int64)
    meta, per_core = host_prep(cfg, x, edge_index, batch)
    nc = build_program(cfg, meta, debug=debug)
    in_maps = make_in_maps(
        cfg, meta, per_core,
        inputs["W_td"], inputs["b_td"], inputs["W_bu"], inputs["b_bu"],
        inputs["fc_W"], inputs["fc_b"],
    )
    return nc, in_maps


def run(cfg, inputs, debug=False, trace=False):
    nc, in_maps = prep_and_build(cfg, inputs, debug=debug)
    res = run_bass_kernel_spmd(nc, in_maps, list(range(cfg.NC)), trace=trace)
    out = res.results[0]["out"].astype(np.float32)
    return out, res


def full_cfg():
    return Cfg(
        n_nodes=100000, n_graphs=1000, n_cores=8, banks=4,
        in_f=128, hid_f=64, out_f=4,
    )


def kernel(**inputs):
    out, _ = run(full_cfg(), inputs)
    return out


# revision 52
# speedup vs baseline: 1.0924x; 1.0924x over previous
"""BiGCN (two fused GCNConv + graph mean-pool + FC + log_softmax) on 8 trn2 cores.

Strategy (graph/data parallel, partitioned by destination node range):
  - core c owns nodes [c*NSH, (c+1)*NSH) as edge destinations
  - host precomputes dinv = 1/sqrt(deg+1) and the transposed x shard, so the
    device does no degree counting and no PE transposes in phase 1
  - device P1: Hn = (xT @ [W_td | W_bu]) * dinv (bf16), computed in batched
    x chunks, kept resident in SBUF, and AllGathered in 4 bank stripes
    (banked so gather indices fit int16)
  - host sorts edges into merged (4-tile-group, bank) gather cells with
    per-core COMPACTED index lists (pads only trail, skipped by the -1
    trim); the per-(tile, bank) one-hots carry tile-local dst rows in
    duplicated boundary-chunk columns, so a 128-slot chunk may span tiles
  - gather descriptor generation is spread over all 4 SWDGE queues (one Q7
    core pair each, ~10 ns/row/pair); the first WIN groups are emitted
    bank-major so each bank's gathers dispatch as soon as its AllGather
    lands; Tile's DMASW lane sems require queue = emission-index % 4 and
    dummy gathers pad the rotation so the (queue-0) indirect scatters land
    on queue-0 lanes
  - per dst tile: one-hot matmul scatter into PSUM, self-loop via identity
    matmul, out = dinv*(sum + Hn) + b; feat = [relu(td),td,relu(bu),bu,1]
  - graph pooling via one-hot matmul (built with a broadcast tensor_tensor),
    indirect-scatter into a bf16 partial, bf16 AllReduce, FC + log_softmax
    replicated on every core.
"""

import math

import numpy as np
import ml_dtypes

import concourse.bass as bass
import concourse.bacc as bacc
import concourse.mybir as mybir
import concourse.tile as tile
from concourse.bass import IndirectOffsetOnAxis
from concourse.bass_utils import run_bass_kernel_spmd
from concourse.library_config import mlp as mlp_lib

BF16 = mybir.dt.bfloat16
F32 = mybir.dt.float32
I16 = mybir.dt.int16
I32 = mybir.dt.int32
AF = mybir.ActivationFunctionType
ALU = mybir.AluOpType
NPBF = ml_dtypes.bfloat16

P = 128  # partitions / tile height

PREP_MODE = False  # prepare_only + trigger_dma pipeline (False: direct gathers)
LEAD = 8  # tiles of desc-gen lead ahead of triggers
PARTIAL_BF16 = True  # pooled partial + AllReduce in bf16
GROUP = 4  # dst tiles merged into one gather instruction per bank
WIN_GROUPS = 4  # gather lookahead window, in groups


def _split_even(n, k):
    base = n // k
    rem = n % k
    return [base + (1 if i < rem else 0) for i in range(k)]


def _split_progressive(n, k, cap):
    """Small first stripes, later stripes at the int16 cap: the first
    AllGather launches after only a few P1 tiles, so the gather stream
    starts early and the collective chain hides under P1 + early gathers."""
    sizes = [0] * k
    rem = n
    for i in range(k - 1, -1, -1):
        take = min(cap, rem - i)
        sizes[i] = take
        rem -= take
    assert rem == 0 and all(s >= 1 for s in sizes)
    return sizes


class Cfg:
    def __init__(self, n_nodes, n_graphs, n_cores, banks, in_f, hid_f, out_f):
        assert n_nodes % n_cores == 0
        self.N = n_nodes
        self.G = n_graphs
        self.NC = n_cores
        self.NSH = n_nodes // n_cores  # nodes per core
        self.T = math.ceil(self.NSH / P)  # dst tiles per core
        self.NSH_P = self.T * P  # padded shard rows
        self.BANKS = min(banks, self.T)
        # bank k holds the stripe of tiles [qt_start[k], qt_start[k+1]) from
        # every core: bank rows = NC * qrows[k]
        # even stripes beat a progressive (small-first) split on HW: tiny
        # early banks starve the Q7 pairs before the next AllGather lands,
        # and max-size late banks coarsen cells and lengthen their AGs
        self.QT = _split_even(self.T, self.BANKS)
        self.QSTART = np.concatenate([[0], np.cumsum(self.QT)])  # tile starts
        self.QROWS = [q * P for q in self.QT]
        for k in range(self.BANKS):
            assert self.NC * self.QROWS[k] <= 32767, "bank idx must fit int16"
        self.IN_F = in_f
        self.HID = hid_f
        self.FW = 2 * hid_f
        assert self.FW == P and in_f == P
        self.OUT_F = out_f
        self.FEAT = 4 * hid_f + 1
        self.GB = math.ceil(self.G / P)
        # rows needed: max goff1+127 = (G-1) + 255
        self.PART_ROWS = math.ceil((self.G - 1 + 2 * P) / P) * P


def host_prep(cfg, x, edge_index, batch):
    """Build per-core edge grids + constants. Returns (meta, per_core_inputs)."""
    c = cfg
    src = edge_index[0].astype(np.int64)
    dst = edge_index[1].astype(np.int64)
    assert src.min() >= 0 and src.max() < c.N and dst.min() >= 0 and dst.max() < c.N

    qstart_rows = c.QSTART[:-1] * P  # local row where each bank stripe starts
    sc = src // c.NSH  # owner core of src
    so = src % c.NSH  # local row of src
    stile = so // P
    bank = np.searchsorted(c.QSTART[1:], stile, side="right")
    lidx = sc * np.asarray(c.QROWS)[bank] + (so - qstart_rows[bank])

    owner = dst // c.NSH
    tloc = (dst % c.NSH) // P
    dl = ((dst % c.NSH) % P).astype(np.int64)

    ncell = c.NC * c.T * c.BANKS
    cell = (owner * c.T + tloc) * c.BANKS + bank
    order = np.argsort(cell, kind="stable")
    cell_s = cell[order]
    lidx_s = lidx[order]
    dl_s = dl[order]
    counts = np.bincount(cell_s, minlength=ncell).reshape(c.NC, c.T, c.BANKS)
    starts = np.zeros(ncell + 1, dtype=np.int64)
    np.cumsum(counts.reshape(-1), out=starts[1:])

    NGRP = math.ceil(c.T / GROUP)

    # host-side degree/dinv (deg includes the self loop)
    deg = np.bincount(dst, minlength=c.N).astype(np.float64) + 1.0
    dinv = (1.0 / np.sqrt(deg)).astype(np.float32)

    g_base = np.empty(c.NC, dtype=np.int64)
    for cc in range(c.NC):
        b = batch[cc * c.NSH : (cc + 1) * c.NSH]
        g_base[cc] = int(b[0])
        assert int(b[-1]) - int(b[0]) < 2 * P, "graph span exceeds 2 blocks"

    # merged (group, bank) gather cells: per-core edges of the group's tiles
    # are COMPACTED (pads only trail, skipped by the -1 trim). A 128-slot
    # chunk may span two tiles; the per-(t, j) one-hot uses group-local dst
    # rows (toff*128 + row) vs an iota512 window to mask foreign slots.
    # Chunk ranges per (t, j) are the union over cores.
    ngc = np.zeros((c.NC, NGRP, c.BANKS), dtype=np.int64)  # edges per core
    for g in range(NGRP):
        t0, t1 = g * GROUP, min((g + 1) * GROUP, c.T)
        ngc[:, g, :] = counts[:, t0:t1, :].sum(axis=1)
    Bg = {}  # (g, j) -> chunks of the merged cell (max over cores)
    ecb = {}  # (g, j) -> eidx chunk col base
    col = 0
    for g in range(NGRP):
        for j in range(c.BANKS):
            ecb[(g, j)] = col
            Bg[(g, j)] = int(-(-(max(int(ngc[:, g, j].max()), 1)) // P))
            col += Bg[(g, j)]
    ECOLS = col * 8
    NMG = NGRP * c.BANKS

    # union chunk range of tile t inside its merged cell, across cores
    crange = {}  # (t, j) -> (clo, chi) chunk indices within the cell
    for g in range(NGRP):
        t0, t1 = g * GROUP, min((g + 1) * GROUP, c.T)
        for j in range(c.BANKS):
            cum = np.zeros(c.NC, dtype=np.int64)
            for t in range(t0, t1):
                nt = counts[:, t, j]
                lo = int(cum.min()) // P
                if int(nt.max()) > 0:
                    hi = -(-int((cum + nt).max()) // P)
                    crange[(t, j)] = (lo, hi)
                else:
                    crange[(t, j)] = (lo, lo)
                cum += nt

    # dl columns ordered (t, j, chunk-in-range); boundary chunks duplicate
    dl_col = {}
    col = 0
    for t in range(c.T):
        for j in range(c.BANKS):
            dl_col[(t, j)] = col
            lo, hi = crange[(t, j)]
            col += hi - lo
    GCH = max(col, 1)

    per_core = []
    scrub_lo = {}  # (g, j) -> lowest chunk any core leaves partially unwritten
    for cc in range(c.NC):
        eidx = np.zeros((P, max(ECOLS, 8)), dtype=np.int16)
        dlh = np.full((P, GCH), 4096.0, dtype=np.float32)
        nreal = np.zeros((1, NMG), dtype=np.int32)
        for g in range(NGRP):
            t0, t1 = g * GROUP, min((g + 1) * GROUP, c.T)
            for j in range(c.BANKS):
                B = Bg[(g, j)]
                slots = B * P
                li = np.full(slots, -1, dtype=np.int64)
                dv = np.full(slots, 4096.0, dtype=np.float64)
                pos = 0
                for t in range(t0, t1):
                    ci = (cc * c.T + t) * c.BANKS + j
                    s0, s1 = starts[ci], starts[ci + 1]
                    n = int(s1 - s0)
                    li[pos : pos + n] = lidx_s[s0:s1]
                    dv[pos : pos + n] = (t - t0) * P + dl_s[s0:s1]
                    pos += n
                if pos == 0:
                    li[0] = 0  # keep >=1 real idx (sim/ucode edge case)
                    pos = 1
                nreal[0, g * c.BANKS + j] = pos
                scrub_lo[(g, j)] = min(scrub_lo.get((g, j), 1 << 30), pos // P)
                w = li.reshape(slots // 16, 16).T.astype(np.int16)
                eb = ecb[(g, j)]
                eidx[:, eb * 8 : eb * 8 + B * 8] = np.tile(w, (8, 1))
                dvc = dv.reshape(B, P).T  # [P, B] chunk columns
                for t in range(t0, t1):
                    lo, hi = crange[(t, j)]
                    if hi > lo:
                        cb = dl_col[(t, j)]
                        # tile-local values: own edges land in [0,128) (bf16
                        # exact); foreign/pad values stay far outside even
                        # after bf16 rounding, so is_equal vs iota128 is safe
                        dlh[:, cb : cb + hi - lo] = (
                            dvc[:, lo:hi] - (t - t0) * P
                        )

        xs = np.zeros((c.NSH_P, c.IN_F), dtype=np.float32)
        xs[: c.NSH] = x[cc * c.NSH : (cc + 1) * c.NSH]
        xT = np.ascontiguousarray(xs.T)  # [IN_F, NSH_P]

        dv = np.ones(c.T * P, dtype=np.float32)
        dv[: c.NSH] = dinv[cc * c.NSH : (cc + 1) * c.NSH]
        dinvT = np.ascontiguousarray(dv.reshape(c.T, P).T)  # [P, T]

        brel = np.full(c.T * P, 60000.0, dtype=np.float32)
        brel[: c.NSH] = batch[cc * c.NSH : (cc + 1) * c.NSH] - g_base[cc]
        batchT = brel.reshape(c.T, P).T.astype(np.float32)

        goff0 = (g_base[cc] + np.arange(P)).astype(np.int32).reshape(P, 1)
        goff1 = goff0 + P
        per_core.append(
            dict(xT_sh=xT, dinvT=dinvT, eidx=eidx, dlh=dlh.astype(NPBF),
                 batchT=batchT, goff0=goff0, goff1=goff1, nreal=nreal)
        )

    iota512 = np.tile(
        np.arange(GROUP * P, dtype=np.float32), (P, 1)
    ).astype(NPBF)
    iota256 = np.tile(np.arange(2 * P, dtype=np.float32), (P, 1)).astype(NPBF)
    ident = np.eye(P, dtype=np.float32).astype(NPBF)
    consts = dict(iota512=iota512, iota256=iota256, ident=ident)
    meta = dict(GCH=GCH, ECOLS=max(ECOLS, 8), consts=consts,
                dl_col=dl_col, ecb=ecb, crange=crange, Bg=Bg, NGRP=NGRP,
                NMG=NMG, scrub_lo=scrub_lo)
    return meta, per_core


def build_program(cfg, meta, debug=False):
    c = cfg
    GCH = meta["GCH"]
    ECOLS = meta["ECOLS"]
    dl_col = meta["dl_col"]
    ecb = meta["ecb"]
    crange = meta["crange"]
    scrub_lo = meta["scrub_lo"]
    Bg = meta["Bg"]
    NGRP = meta["NGRP"]
    NMG = meta["NMG"]
    H = c.HID
    PD = BF16 if PARTIAL_BF16 else F32
    LEAD_G = max(1, LEAD // GROUP)  # gather lead in groups (prep mode)
    WIN = WIN_GROUPS  # gather window, in groups
    GAT_BUFS = WIN * c.BANKS  # in-flight merged gather tiles
    BMAX = max(Bg.values()) if Bg else 1  # widest merged cell (chunks)

    nc = bacc.Bacc(
        "TRN2", target_bir_lowering=False, debug=debug, num_devices=c.NC,
        num_swdge_queues=min(4, c.BANKS),
    )

    # ---- I/O ----
    xT_sh = nc.dram_tensor("xT_sh", [c.IN_F, c.NSH_P], F32, kind="ExternalInput")
    dinvT = nc.dram_tensor("dinvT", [P, c.T], F32, kind="ExternalInput")
    W_td = nc.dram_tensor("W_td", [c.IN_F, H], F32, kind="ExternalInput")
    W_bu = nc.dram_tensor("W_bu", [c.IN_F, H], F32, kind="ExternalInput")
    b_td = nc.dram_tensor("b_td", [H], F32, kind="ExternalInput")
    b_bu = nc.dram_tensor("b_bu", [H], F32, kind="ExternalInput")
    fc_W = nc.dram_tensor("fc_W", [4 * H, c.OUT_F], F32, kind="ExternalInput")
    fc_b = nc.dram_tensor("fc_b", [c.OUT_F], F32, kind="ExternalInput")
    eidx = nc.dram_tensor("eidx", [P, ECOLS], I16, kind="ExternalInput")
    dlh = nc.dram_tensor("dlh", [P, max(GCH, 1)], BF16, kind="ExternalInput")
    batchT = nc.dram_tensor("batchT", [P, c.T], F32, kind="ExternalInput")
    goff0 = nc.dram_tensor("goff0", [P, 1], I32, kind="ExternalInput")
    goff1 = nc.dram_tensor("goff1", [P, 1], I32, kind="ExternalInput")
    nreal = nc.dram_tensor("nreal", [1, NMG], I32, kind="ExternalInput")
    iota512_in = nc.dram_tensor(
        "iota512", [P, GROUP * P], BF16, kind="ExternalInput"
    )
    iota256_in = nc.dram_tensor("iota256", [P, 2 * P], BF16, kind="ExternalInput")
    ident_in = nc.dram_tensor("ident", [P, P], BF16, kind="ExternalInput")
    out = nc.dram_tensor("out", [c.G, c.OUT_F], F32, kind="ExternalOutput")

    # ---- internal DRAM ----
    hn_local = nc.dram_tensor("hn_local", [c.NSH_P, c.FW], BF16)
    hn_q = [
        nc.dram_tensor(f"hn_q{k}", [c.NC * c.QROWS[k], c.FW], BF16,
                       addr_space="Shared")
        for k in range(c.BANKS)
    ]
    partial = nc.dram_tensor("partial", [c.PART_ROWS, c.FEAT], PD)
    total = nc.dram_tensor("total", [c.PART_ROWS, c.FEAT], PD, addr_space="Shared")

    groups = [list(range(c.NC))]
    NQ = min(4, c.BANKS)  # swdge queues in use (queue = bank % NQ)

    with tile.TileContext(nc) as tc:
        with (
            tc.tile_pool(name="const", bufs=1) as cp,
            tc.tile_pool(name="sb", bufs=3) as sp,
            tc.tile_pool(name="ohb", bufs=2) as op_,
            tc.tile_pool(name="gat", bufs=WIN_GROUPS * c.BANKS) as gp,
            nc.gpsimd.register("nr0") as r0,
            nc.gpsimd.register("nr1") as r1,
            nc.gpsimd.register("nr2") as r2,
            nc.gpsimd.register("nr3") as r3,
        ):
            regs = [r0, r1, r2, r3]
            nc.gpsimd.load_library(mlp_lib)

            dsem = [nc.alloc_semaphore(f"gdma_q{j}") for j in range(NQ)]

            # ---- constants ----
            # P1-critical loads go on the sync HWDGE queue; bulk loads that
            # are only needed by the gather phase ride the scalar HWDGE
            # queue so they don't delay the first x chunks / AllGather
            wtmp = cp.tile([P, c.FW], F32)
            nc.sync.dma_start(wtmp[:, 0:H], W_td[:])
            nc.sync.dma_start(wtmp[:, H : 2 * H], W_bu[:])
            wcat = cp.tile([P, c.FW], BF16)
            nc.scalar.activation(wcat[:], wtmp[:], AF.Copy)
            dinv_sb = cp.tile([P, c.T], F32)
            nc.sync.dma_start(dinv_sb[:], dinvT[:])

            eidx_sb = cp.tile([P, ECOLS], I16)
            nc.scalar.dma_start(eidx_sb[:], eidx[:])

            iota_sb = cp.tile([P, GROUP * P], BF16)
            iota256_sb = cp.tile([P, 2 * P], BF16)
            ident_sb = cp.tile([P, P], BF16)
            nc.scalar.dma_start(iota_sb[:], iota512_in[:])
            nc.scalar.dma_start(iota256_sb[:], iota256_in[:])
            nc.scalar.dma_start(ident_sb[:], ident_in[:])
            ident32_sb = cp.tile([P, P], F32)
            nc.scalar.activation(ident32_sb[:], ident_sb[:], AF.Copy)

            btmp = cp.tile([1, c.FW], F32)
            nc.scalar.dma_start(btmp[0:1, 0:H], b_td[None, :])
            nc.scalar.dma_start(btmp[0:1, H : 2 * H], b_bu[None, :])
            bcat = cp.tile([1, c.FW], BF16)
            nc.scalar.activation(bcat[:], btmp[:], AF.Copy)
            ones_row = cp.tile([1, P], BF16)
            nc.vector.memset(ones_row[:], 1.0)
            bias_sb = cp.tile([P, c.FW], F32)

            dl_sb = cp.tile([P, max(GCH, 1)], BF16)
            nc.scalar.dma_start(dl_sb[:], dlh[:])
            nreal_sb = cp.tile([1, NMG], I32)
            nc.scalar.dma_start(nreal_sb[:], nreal[:])

            fw0 = cp.tile([P, c.OUT_F], F32)
            fw1 = cp.tile([P, c.OUT_F], F32)
            fcb = cp.tile([c.OUT_F, 1], F32)
            batch_sb = cp.tile([P, c.T], F32)
            goff0_sb = cp.tile([P, 1], I32)
            goff1_sb = cp.tile([P, 1], I32)
            hn_all = cp.tile([P, c.T * P], BF16)  # resident Hn tiles
            zt = sp.tile([P, c.FEAT], PD, tag="zt")

            def emit_late_loads():
                # needed only by the pooling epilogue / FC — emitted after
                # the gather window so they don't delay phase 1
                nc.scalar.dma_start(fw0[:], fc_W[0:P, :])
                nc.scalar.dma_start(fw1[:], fc_W[P : 2 * P, :])
                nc.scalar.dma_start(fcb[:, 0:1], fc_b[:, None])
                nc.scalar.dma_start(batch_sb[:], batchT[:])
                nc.scalar.dma_start(goff0_sb[:], goff0[:])
                nc.scalar.dma_start(goff1_sb[:], goff1[:])
                nc.vector.memset(zt[:], 0.0)
                for r in range(0, c.PART_ROWS, P):
                    nc.scalar.dma_start(partial[r : r + P, :], zt[:])

            # ---- gather machinery (one merged gather per (group, bank)) ----
            gts = {}  # (g, j) -> gather output tile
            prep_done = cp.tile([P, 8], F32)  # all-desc-gen-done marker
            acnt = [0]  # allocation counter (first-use scrub)
            ecnt = [0]  # emission counter (queue + reg round-robin)

            def alloc_cell(g, j):
                # pool slots are assigned in allocation order — keep that
                # aligned with (g, j) consumption order so slot WAR waits
                # stay one window behind
                B = int(Bg[(g, j)])
                if B == 0:
                    return
                # uniform-width tiles: slot reuse must never expose SBUF the
                # first-use scrub didn't cover (trailing pad slots are
                # skipped by the gather's -1 trim and reach the matmuls
                # zero-weighted, so they must be finite, not stale NaNs)
                gt_t = gp.tile([P, BMAX * P], BF16, tag="gt")
                gts[(g, j)] = gt_t
                if acnt[0] < GAT_BUFS:
                    # first slot use: whole tile may be NaN garbage
                    nc.vector.memset(gt_t[:], 0.0)
                else:
                    # the -1 trim leaves slots beyond this core's real count
                    # unwritten; scrub from the lowest chunk any core can
                    # leave partial up to this cell's width (stale finite
                    # data would be fine, but slot-width variance across
                    # reuses can expose never-written SBUF)
                    lo = min(scrub_lo[(g, j)], B - 1)
                    nc.vector.memset(gt_t[:, lo * P : B * P], 0.0)
                acnt[0] += 1

            def gather_cell(g, j):
                B = int(Bg[(g, j)])
                if B == 0:
                    return
                q = ecnt[0] % NQ
                gt_t = gts[(g, j)]
                eb = ecb[(g, j)]
                mg = g * c.BANKS + j
                reg = regs[ecnt[0] % 4]
                nc.gpsimd.reg_load(reg, nreal_sb[0:1, mg : mg + 1])
                kwargs = dict(queue_num=q, single_packet=(B * P <= 1024))
                if PREP_MODE:
                    kwargs.update(prepare_only=True, sem=dsem[q])
                nc.gpsimd.dma_gather(
                    gt_t[:, 0 : B * P].rearrange("p (b e) -> p b e", e=P),
                    hn_q[j][:],
                    eidx_sb[:, eb * 8 : eb * 8 + B * 8],
                    B * P,
                    reg,
                    c.FW,
                    **kwargs,
                )
                ecnt[0] += 1

            def emit_preps(g):
                for j in range(c.BANKS):
                    alloc_cell(g, j)
                    gather_cell(g, j)

            def emit_triggers(g, tail=False):
                if not PREP_MODE:
                    return
                if tail:
                    # tail triggers have no pending preps, so no nosync deps
                    # hold them in place — pin the schedule order explicitly
                    tc.no_sync_barrier()
                for j in range(c.BANKS):
                    if Bg[(g, j)] == 0:
                        continue
                    q = j % NQ
                    if tail:
                        # WAW on prep_done (written by the all-cores gpsimd
                        # memset after the last prep) orders the trigger
                        # after every pair's desc-gen has completed
                        nc.gpsimd.trigger_dma(
                            count=1, queue_num=q,
                            signals_writable=[prep_done[:]],
                        )
                    else:
                        # count=None fires the FIFO head; Tile attaches the
                        # pending prep's engine-completion wait + deferred
                        # data deps (hn_q AllGather, gt-slot WAR)
                        nc.gpsimd.trigger_dma(count=None, queue_num=q)

            # ---- prologue: desc-gen for the first LEAD_G groups ----
            # (prep mode only: direct gathers carry data deps on the
            # AllGathers, which are emitted later in the Pool stream —
            # hoisting them here would deadlock the in-order sequencer)
            if PREP_MODE:
                for g in range(min(LEAD_G, NGRP)):
                    emit_preps(g)
                if NGRP <= LEAD_G:
                    nc.gpsimd.memset(prep_done[:], 0.0)

            # ---- P1: Hn = (xT @ wcat) * dinv, then banked AllGathers ----
            CH = min(8, c.T)  # tiles per x chunk
            with (
                tc.tile_pool(name="p1x", bufs=2) as xp,
                tc.tile_pool(name="ps1", bufs=4, space="PSUM") as pp,
            ):
                bias_ps = pp.tile([P, c.FW], F32, space="PSUM", tag="bias")
                nc.tensor.matmul(
                    bias_ps[:], lhsT=ones_row[0:1, :], rhs=bcat[0:1, :],
                    start=True, stop=True,
                )
                nc.vector.tensor_copy(bias_sb[:], bias_ps[:])

                for k in range(c.BANKS):
                    for t0 in range(int(c.QSTART[k]), int(c.QSTART[k + 1]), CH):
                        t1 = min(t0 + CH, int(c.QSTART[k + 1]))
                        w = (t1 - t0) * P
                        xf = xp.tile([P, CH * P], F32, tag="xf")
                        nc.sync.dma_start(xf[:, 0:w], xT_sh[:, t0 * P : t1 * P])
                        xb = xp.tile([P, CH * P], BF16, tag="xb")
                        nc.scalar.activation(xb[:, 0:w], xf[:, 0:w], AF.Copy)
                        for t in range(t0, t1):
                            h_ps = pp.tile([P, c.FW], F32, space="PSUM", tag="h")
                            nc.tensor.matmul(
                                h_ps[:],
                                lhsT=xb[:, (t - t0) * P : (t - t0 + 1) * P],
                                rhs=wcat[:],
                                start=True, stop=True,
                            )
                            nc.scalar.activation(
                                hn_all[:, t * P : (t + 1) * P], h_ps[:],
                                AF.Copy, scale=dinv_sb[:, t : t + 1],
                            )
                            nc.sync.dma_start(
                                hn_local[t * P : (t + 1) * P, :],
                                hn_all[:, t * P : (t + 1) * P],
                            )

                    r_lo = int(c.QSTART[k]) * P
                    nc.gpsimd.collective_compute(
                        "AllGather",
                        ALU.bypass,
                        ins=[hn_local[r_lo : r_lo + c.QROWS[k], :]],
                        outs=[hn_q[k][:]],
                        replica_groups=groups,
                    )

            # ---- initial gather window (direct mode): bank-major so each
            # bank's first gathers dispatch as soon as its AllGather lands,
            # keeping all queues fed while later AllGathers finish ----
            if not PREP_MODE:
                for g in range(min(WIN, NGRP)):
                    for j in range(c.BANKS):
                        alloc_cell(g, j)
                for j in range(c.BANKS):
                    for g in range(min(WIN, NGRP)):
                        gather_cell(g, j)
            emit_late_loads()

            # ---- P4: trigger + scatter-add + feat + pooling ----
            def onehot_big(t, tag):
                # dl columns hold tile-local dst rows; foreign-tile and pad
                # slots sit far outside [0, 128) so they never match
                g0 = dl_col[(t, 0)]
                gt = sum(
                    crange[(t, j)][1] - crange[(t, j)][0]
                    for j in range(c.BANKS)
                )
                oh = op_.tile([P, gt * P], BF16, tag=tag)
                nc.vector.tensor_tensor(
                    out=oh[:].rearrange("p (g d) -> p g d", d=P),
                    in0=iota_sb[:, 0:P].unsqueeze(1).broadcast_to([P, gt, P]),
                    in1=dl_sb[:, g0 : g0 + gt].to_broadcast([P, gt, P]),
                    op=ALU.is_equal,
                )
                return oh, g0, gt

            with (
                tc.tile_pool(name="ps4", bufs=2, space="PSUM") as pp,
                tc.tile_pool(name="psacc", bufs=1, space="PSUM") as pa,
            ):
                pool_ps0 = pa.tile([P, c.FEAT], F32, space="PSUM")
                pool_ps1 = pa.tile([P, c.FEAT], F32, space="PSUM")
                for t in range(c.T):
                    g = t // GROUP
                    if t % GROUP == 0:
                        if PREP_MODE:
                            if g + LEAD_G < NGRP:
                                emit_preps(g + LEAD_G)
                                if g + LEAD_G == NGRP - 1:
                                    # marker after the final prep: the
                                    # all-cores memset completes only once
                                    # every Q7 pair finished its desc-gen
                                    nc.gpsimd.memset(prep_done[:], 0.0)
                            emit_triggers(g, tail=(g + LEAD_G >= NGRP))
                        elif g + WIN < NGRP:
                            emit_preps(g + WIN)

                    tcells = [
                        (j, dl_col[(t, j)], crange[(t, j)])
                        for j in range(c.BANKS)
                        if crange[(t, j)][1] > crange[(t, j)][0]
                    ]
                    nch = sum(hi - lo for _, _, (lo, hi) in tcells)
                    acc = pp.tile([P, c.FW], F32, space="PSUM", tag="acc")
                    if nch:
                        oh, g0, gtn = onehot_big(t, "ohb2")
                        kk = 0
                        for j, cb, (lo, hi) in tcells:
                            gt_t = gts[(g, j)]
                            for q in range(hi - lo):
                                nc.tensor.matmul(
                                    acc[:],
                                    lhsT=oh[:, (cb - g0 + q) * P
                                            : (cb - g0 + q + 1) * P],
                                    rhs=gt_t[:, (lo + q) * P
                                             : (lo + q + 1) * P],
                                    start=(kk == 0),
                                    stop=False,
                                )
                                kk += 1
                    nc.tensor.matmul(
                        acc[:], lhsT=ident_sb[:],
                        rhs=hn_all[:, t * P : (t + 1) * P],
                        start=(nch == 0), stop=True,
                    )

                    ot = sp.tile([P, c.FW], F32, tag="ot")
                    nc.scalar.activation(
                        ot[:], acc[:], AF.Copy, scale=dinv_sb[:, t : t + 1]
                    )
                    nc.vector.tensor_tensor(
                        out=ot[:], in0=ot[:], in1=bias_sb[:], op=ALU.add
                    )
                    feat = sp.tile([P, c.FEAT], BF16, tag="feat")
                    nc.scalar.activation(feat[:, 0:H], ot[:, 0:H], AF.Relu)
                    nc.scalar.copy(feat[:, H : 2 * H], ot[:, 0:H])
                    nc.scalar.activation(
                        feat[:, 2 * H : 3 * H], ot[:, H : 2 * H], AF.Relu
                    )
                    nc.scalar.copy(feat[:, 3 * H : 4 * H], ot[:, H : 2 * H])
                    nc.vector.memset(feat[:, 4 * H : 4 * H + 1], 1.0)

                    ohg = sp.tile([P, 2 * P], BF16, tag="ohg")
                    nc.vector.tensor_tensor(
                        out=ohg[:].rearrange("p (g d) -> p g d", d=2 * P),
                        in0=iota256_sb[:].unsqueeze(1),
                        in1=batch_sb[:, t : t + 1].to_broadcast([P, 1, 2 * P]),
                        op=ALU.is_equal,
                    )
                    nc.tensor.matmul(
                        pool_ps0[:], lhsT=ohg[:, 0:P], rhs=feat[:],
                        start=(t == 0), stop=(t == c.T - 1),
                    )
                    nc.tensor.matmul(
                        pool_ps1[:], lhsT=ohg[:, P : 2 * P], rhs=feat[:],
                        start=(t == 0), stop=(t == c.T - 1),
                    )

                # ---- P5: scatter local pooled windows ----
                # Tile's DMASW lane sems rotate over Pool DMA instructions
                # and each lane is locked to one SWDGE queue. The indirect
                # scatters are pinned to queue 0, so pad the rotation with
                # dummy gathers to land them on queue-0 lanes (0 and 4);
                # the barrier keeps the no-dep dummies from being hoisted.
                def dummy_gather(scrap):
                    tc.no_sync_barrier()
                    q = ecnt[0] % NQ
                    nc.gpsimd.dma_gather(
                        scrap[:].rearrange("p (b e) -> p b e", e=P),
                        hn_q[0][:],
                        eidx_sb[:, 0:8],
                        P,
                        P,
                        c.FW,
                        queue_num=q,
                        single_packet=True,
                    )
                    ecnt[0] += 1

                tc.no_sync_barrier()
                scrap = sp.tile([P, P], BF16, tag="scrap")
                while ecnt[0] % 4 != 0:
                    dummy_gather(scrap)
                pp0 = sp.tile([P, c.FEAT], PD, tag="pp0")
                nc.vector.tensor_copy(pp0[:], pool_ps0[:])
                tc.no_sync_barrier()
                nc.gpsimd.indirect_dma_start(
                    out=partial[:],
                    out_offset=IndirectOffsetOnAxis(ap=goff0_sb[:, 0:1], axis=0),
                    in_=pp0[:],
                    in_offset=None,
                )
                ecnt[0] += 1
                for _ in range(3):
                    dummy_gather(scrap)
                pp1 = sp.tile([P, c.FEAT], PD, tag="pp1")
                nc.vector.tensor_copy(pp1[:], pool_ps1[:])
                tc.no_sync_barrier()
                nc.gpsimd.indirect_dma_start(
                    out=partial[:],
                    out_offset=IndirectOffsetOnAxis(ap=goff1_sb[:, 0:1], axis=0),
                    in_=pp1[:],
                    in_offset=None,
                )

            # ---- P6: AllReduce pooled sums ----
            nc.gpsimd.collective_compute(
                "AllReduce",
                ALU.add,
                ins=[partial[:]],
                outs=[total[:]],
                replica_groups=groups,
            )

            # ---- P7: mean, FC, log_softmax (replicated) ----
            with tc.tile_pool(name="ps7", bufs=2, space="PSUM") as pp:
                for b in range(c.GB):
                    h_rows = min(P, c.G - b * P)
                    tt = sp.tile([P, c.FEAT], PD, tag="tt")
                    nc.sync.dma_start(tt[:], total[b * P : (b + 1) * P, :])
                    rec = sp.tile([P, 1], F32, tag="rec")
                    nc.vector.tensor_scalar(
                        out=rec[:], in0=tt[:, 4 * H : 4 * H + 1], scalar1=1.0,
                        scalar2=None, op0=ALU.max,
                    )
                    nc.vector.reciprocal(rec[:], rec[:])
                    mean_sb = sp.tile([P, 4 * H], F32, tag="mean")
                    nc.vector.tensor_scalar(
                        out=mean_sb[:], in0=tt[:, 0 : 4 * H],
                        scalar1=rec[:, 0:1], scalar2=None, op0=ALU.mult,
                    )
                    lg_ps = pp.tile([P, P], F32, space="PSUM", tag="lg")
                    for half in range(2):
                        tp_ps = pp.tile([P, P], F32, space="PSUM", tag="tp")
                        nc.tensor.transpose(
                            tp_ps[:], mean_sb[:, half * P : (half + 1) * P],
                            ident32_sb[:],
                        )
                        mt = sp.tile([P, P], F32, tag="mt")
                        nc.vector.tensor_copy(mt[:], tp_ps[:])
                        nc.tensor.matmul(
                            lg_ps[0 : c.OUT_F, :],
                            lhsT=(fw0 if half == 0 else fw1)[:],
                            rhs=mt[:],
                            start=(half == 0),
                            stop=(half == 1),
                        )
                    lgb = sp.tile([c.OUT_F, P], F32, tag="lgb")
                    nc.vector.tensor_scalar(
                        out=lgb[:], in0=lg_ps[0 : c.OUT_F, :],
                        scalar1=fcb[:, 0:1], scalar2=None, op0=ALU.add,
                    )
                    tr_ps = pp.tile([P, c.OUT_F], F32, space="PSUM", tag="tr")
                    nc.tensor.transpose(
                        tr_ps[:], lgb[:], ident32_sb[0 : c.OUT_F, 0 : c.OUT_F]
                    )
                    ls = sp.tile([P, c.OUT_F], F32, tag="ls")
                    nc.vector.tensor_copy(ls[:], tr_ps[:])
                    mx = sp.tile([P, 1], F32, tag="mx")
                    nc.vector.reduce_max(mx[:], ls[:], axis=mybir.AxisListType.X)
                    nc.vector.tensor_scalar(
                        out=ls[:], in0=ls[:], scalar1=mx[:, 0:1], scalar2=None,
                        op0=ALU.subtract,
                    )
                    ex = sp.tile([P, c.OUT_F], F32, tag="ex")
                    nc.scalar.activation(ex[:], ls[:], AF.Exp)
                    sm = sp.tile([P, 1], F32, tag="sm")
                    nc.vector.reduce_sum(sm[:], ex[:], axis=mybir.AxisListType.X)
                    nc.scalar.activation(sm[:], sm[:], AF.Ln)
                    nc.vector.tensor_scalar(
                        out=ls[:], in0=ls[:], scalar1=sm[:, 0:1], scalar2=None,
                        op0=ALU.subtract,
                    )
                    nc.sync.dma_start(
                        out[b * P : b * P + h_rows, :], ls[0:h_rows, :]
                    )

    nc.compile()
    return nc


def make_in_maps(cfg, meta, per_core, W_td, b_td, W_bu, b_bu, fc_W, fc_b):
    cst = meta["consts"]
    in_maps = []
    for cc in range(cfg.NC):
        pc = per_core[cc]
        in_maps.append(
            {
                "xT_sh": pc["xT_sh"],
                "dinvT": pc["dinvT"],
                "W_td": np.asarray(W_td, dtype=np.float32),
                "W_bu": np.asarray(W_bu, dtype=np.float32),
                "b_td": np.asarray(b_td, dtype=np.float32),
                "b_bu": np.asarray(b_bu, dtype=np.float32),
                "fc_W": np.asarray(fc_W, dtype=np.float32),
                "fc_b": np.asarray(fc_b, dtype=np.float32),
                "eidx": pc["eidx"],
                "dlh": pc["dlh"],
                "batchT": pc["batchT"],
                "goff0": pc["goff0"],
                "goff1": pc["goff1"],
                "nreal": pc["nreal"],
                "iota512": cst["iota512"],
                "iota256": cst["iota256"],
                "ident": cst["ident"],
            }
        )
    return in_maps


def prep_and_build(cfg, inputs, debug=False):
    x = np.asarray(inputs["x"], dtype=np.float32)
    edge_index = np.asarray(inputs["edge_index"])
    batch = np.asarray(inputs["batch"]).astype(np.int64)
    meta, per_core = host_prep(cfg, x, edge_index, batch)
    nc = build_program(cfg, meta, debug=debug)
    in_maps = make_in_maps(
        cfg, meta, per_core,
        inputs["W_td"], inputs["b_td"], inputs["W_bu"], inputs["b_bu"],
        inputs["fc_W"], inputs["fc_b"],
    )
    return nc, in_maps


def run(cfg, inputs, debug=False, trace=False):
    nc, in_maps = prep_and_build(cfg, inputs, debug=debug)
    res = run_bass_kernel_spmd(nc, in_maps, list(range(cfg.NC)), trace=trace)
    out = res.results[0]["out"].astype(np.float32)
    return out, res


def full_cfg():
    return Cfg(
        n_nodes=100000, n_graphs=1000, n_cores=8, banks=4,
        in_f=128, hid_f=64, out_f=4,
    )


def kernel(**inputs):
    out, _ = run(full_cfg(), inputs)
    return out
